# revision 57
# baseline (speedup 1.0000x reference)
"""Trainium2 Bass kernel for the autoregressive GRU decoder.

Problem: 512 sequential GRU steps over batch 4096, hidden 128; per step the
3-dim position output feeds back into the input.  Data-parallel over 8 cores
(512 batch rows per core), with the per-core batch split into 2 streams of
256 so the engines can pipeline across the sequential dependency chain.

Algebraic restructuring done on the host (validated vs fp64 golden):
  - pos_k = W_out h_k + b_out feeds the next step's input, so the input
    matmul folds into the hidden one: W_c = W_ih[:, :3] @ W_out + W_hh, with
    all z / bias contributions collapsed into one constant C per gate.
  - The gate pre-activations live persistently in PSUM.  They are
    initialized once with C (via an identity matmul) + W @ h, and every
    subsequent step only accumulates W_c @ delta where
    delta = h_new - h = (u - 1) * (h - n).  No per-step constant re-adds.
  - pos is likewise delta-accumulated, transposed: posT[3, 512] persistent
    in PSUM, += W_out @ delta each step (tiny constant stationary operand),
    snapshotted [3, 512] per step into SBUF and staged to DRAM; a final
    on-chip transpose pass (TensorE) produces the [b, t, i] output layout.

Layout per core: hidden state transposed hT [128(H), 512(B)] in SBUF, fp32.
PSUM banks: rz_A, nx_A, rz_B, nx_B (gate preacts, [r|z] / [xn|hn] halves),
posT, 2 transpose banks.
"""

import os
import numpy as np

B, H, LAT, IN = 4096, 128, 32, 3
NCORES = 8
BSH = B // NCORES          # 512 batch rows per core
NST = 2                    # streams per core
BST = BSH // NST           # 256 batch rows per stream
TSNAP = 16                 # pos snapshots buffered in SBUF between DMAs

# "f32" | "f32r" | "bf16" for the per-step delta matmuls (gates + pos);
# the init matmuls always run exact f32.
MM_DTYPE = os.environ.get("KERNEL_MM_DTYPE", "bf16")
# elementwise working dtype for t/s/n/ru/dd tiles: "f32" | "bf16"
EW_DTYPE = os.environ.get("KERNEL_EW_DTYPE", "bf16")

_CACHE = {}


def _host_prep(context, z, W_ih, W_hh, b_ih, b_hh, W_out, b_out):
    """Fold weights/constants; build per-core input maps."""
    f32 = np.float32
    sl = [slice(0, H), slice(H, 2 * H), slice(2 * H, 3 * H)]  # r, z, n rows

    Wp = (W_ih[:, :IN] @ W_out).astype(f32)           # pos feedback fold
    C0 = (W_ih[:, IN:] @ z.T + b_ih[:, None]).astype(f32)     # [384, B]
    C1 = (C0 + (W_ih[:, :IN] @ b_out)[:, None]).astype(f32)

    Wc_r = (Wp[sl[0]] + W_hh[sl[0]]).astype(f32)
    Wc_z = (Wp[sl[1]] + W_hh[sl[1]]).astype(f32)
    Wc_xn = Wp[sl[2]].astype(f32)
    W_hn = W_hh[sl[2]].astype(f32)

    def asc(a):
        return np.ascontiguousarray(a, dtype=f32)

    weights = {
        "w_r": asc(Wc_r.T), "w_z": asc(-Wc_z.T),
        "w_xn": asc(Wc_xn.T), "w_hn": asc(W_hn.T),
        "w0_r": asc(W_hh[sl[0]].T), "w0_z": asc(-W_hh[sl[1]].T),
        "w_out_t": asc(W_out.T),                       # [128, 3]
        "ident": np.eye(H, dtype=f32),
        "c_pos": asc(np.broadcast_to(b_out[:, None], (IN, BSH))),
    }

    in_maps = []
    for c in range(NCORES):
        bs = slice(c * BSH, (c + 1) * BSH)
        m = dict(weights)
        m["h0t"] = asc(context[bs].T)                  # [H, BSH]
        for X in range(NST):
            cs = slice(c * BSH + X * BST, c * BSH + (X + 1) * BST)
            bhh = b_hh[:, None]
            m[f"c0_rz_{X}"] = asc(np.concatenate(
                [C0[sl[0], cs] + bhh[sl[0]],
                 -(C0[sl[1], cs] + bhh[sl[1]])], axis=1))
            m[f"c1_rz_{X}"] = asc(np.concatenate(
                [C1[sl[0], cs] + bhh[sl[0]],
                 -(C1[sl[1], cs] + bhh[sl[1]])], axis=1))
            hn_const = np.broadcast_to(b_hh[sl[2]][:, None], (H, BST))
            m[f"c0_nx_{X}"] = asc(np.concatenate([C0[sl[2], cs], hn_const], axis=1))
            m[f"c1_nx_{X}"] = asc(np.concatenate([C1[sl[2], cs], hn_const], axis=1))
        in_maps.append(m)
    return in_maps


def _build(steps):
    import concourse.bacc as bacc
    import concourse.mybir as mybir
    from concourse.tile import TileContext

    f32 = mybir.dt.float32
    bf16 = mybir.dt.bfloat16
    Act = mybir.ActivationFunctionType
    Op = mybir.AluOpType

    ew_dt = bf16 if EW_DTYPE == "bf16" else f32
    # dtype for the per-step delta matmul operands (weights + delta).
    mm_dt = {"f32": f32, "f32r": mybir.dt.float32r, "bf16": bf16}[MM_DTYPE]

    nc = bacc.Bacc("TRN2", target_bir_lowering=False, debug=False)

    # ---- DRAM parameters ----
    names_2d = ["h0t"] + [f"c{i}_{g}_{X}" for i in (0, 1)
                          for g in ("rz", "nx") for X in range(NST)]
    params = {}
    for n in names_2d:
        params[n] = nc.declare_dram_parameter(n, [H, BSH], f32, isOutput=False)
    for n in ("w_r", "w_z", "w_xn", "w_hn", "w0_r", "w0_z", "ident"):
        params[n] = nc.declare_dram_parameter(n, [H, H], f32, isOutput=False)
    params["w_out_t"] = nc.declare_dram_parameter("w_out_t", [H, IN], f32,
                                                  isOutput=False)
    params["c_pos"] = nc.declare_dram_parameter("c_pos", [IN, BSH], f32,
                                                isOutput=False)
    p_out = nc.declare_dram_parameter("out", [BSH, steps, IN], f32,
                                      isOutput=True)
    # staging in DRAM for posT snapshots, [t, i, b]
    stage = nc.dram_tensor("pos_stage", [steps, IN, BSH], f32)

    with TileContext(nc) as tc, \
            tc.tile_pool(name="const", bufs=1) as cpool, \
            tc.tile_pool(name="state", bufs=1) as spool, \
            tc.tile_pool(name="work", bufs=3) as wpool, \
            tc.tile_pool(name="stage", bufs=2) as stpool, \
            tc.tile_pool(name="psum", bufs=1, space="PSUM") as ppool:

        sb = {}
        for n, p in params.items():
            t = cpool.tile(list(p.shape), f32, name=f"sb_{n}")
            nc.sync.dma_start(out=t[:], in_=p[:])
            sb[n] = t

        # per-step matmul weights, converted to the mm dtype once
        stepw = {}
        for n in ("w_r", "w_z", "w_xn", "w_hn", "w_out_t"):
            if MM_DTYPE != "f32":
                t = cpool.tile(list(params[n].shape), mm_dt, name=f"cw_{n}")
                nc.vector.tensor_copy(t[:], sb[n][:])
                stepw[n] = t
            else:
                stepw[n] = sb[n]

        h = spool.tile([H, BSH], f32, name="h")
        nc.sync.dma_start(out=h[:], in_=params["h0t"][:])

        rz = [ppool.tile([128, 2 * BST], f32, name=f"ps_rz{X}")
              for X in range(NST)]
        nx = [ppool.tile([128, 2 * BST], f32, name=f"ps_nx{X}")
              for X in range(NST)]
        post = ppool.tile([IN, BSH], f32, name="ps_post")
        # scratch bank + const source for PE warm-up filler matmuls
        scratch = ppool.tile([128, 512], f32, name="ps_scratch")
        dummy_src = cpool.tile([128, 512], mm_dt, name="dummy_src")
        nc.vector.tensor_copy(dummy_src[:], sb["h0t"][:])

        # posT init: b_out + W_out h_0   (exact f32)
        nc.tensor.matmul(post[:], sb["ident"][0:IN, 0:IN], sb["c_pos"][:],
                         start=True, stop=False)
        nc.tensor.matmul(post[:], sb["w_out_t"][:], h[:],
                         start=False, stop=True)

        delta_prev = [None, None]
        stg = [None]
        stagger = {}

        def emit_pos(dja):
            """posT += W_out @ delta_joint (one N=512 matmul; pos is off the
            recurrence critical path so cross-stream coupling is harmless)."""
            nc.tensor.matmul(post[:], stepw["w_out_t"][:], dja[:],
                             start=False, stop=True)

        def emit_snap(k):
            """Snapshot posT (state of step k) into the SBUF staging buffer
            and DMA a TSNAP-step block out when full."""
            kloc = k % TSNAP
            if kloc == 0:
                stg[0] = stpool.tile([IN, TSNAP * BSH], f32, name="stg")
            nc.vector.tensor_copy(stg[0][:, kloc * BSH:(kloc + 1) * BSH],
                                  post[:])
            if kloc == TSNAP - 1:
                t0 = k - kloc
                nc.sync.dma_start(
                    out=stage[t0:t0 + TSNAP, :, :].rearrange(
                        "t p b -> p t b"),
                    in_=stg[0][:].rearrange("p (t b) -> p t b", t=TSNAP))

        from concourse.tile_rust import add_dep_helper

        def emit_gates(k, mode, X):
            """Gate matmuls + pos for one stream; returns the first matmul
            (for the anti-phase dependency ladder)."""
            first = None
            if mode < 2:
                ci = f"c{mode}"
                first = nc.tensor.matmul(rz[X][:], sb["ident"][:],
                                         sb[f"{ci}_rz_{X}"][:],
                                         start=True, stop=False)
                nc.tensor.matmul(nx[X][:], sb["ident"][:],
                                 sb[f"{ci}_nx_{X}"][:],
                                 start=True, stop=(mode == 0))
                wr = sb["w0_r"] if mode == 0 else sb["w_r"]
                wz = sb["w0_z"] if mode == 0 else sb["w_z"]
                gate_mms = [(wr, rz, 0, False), (wz, rz, 1, True),
                            (sb["w_hn"], nx, 1, True)]
                if mode == 1:
                    gate_mms.insert(2, (sb["w_xn"], nx, 0, False))
                for w, bank, half, stop in gate_mms:
                    nc.tensor.matmul(
                        bank[X][:, half * BST:(half + 1) * BST],
                        w[:], h[:, X * BST:(X + 1) * BST],
                        start=False, stop=stop)
            else:
                for w, bank, half, stop in [
                        (stepw["w_r"], rz, 0, False),
                        (stepw["w_z"], rz, 1, True),
                        (stepw["w_xn"], nx, 0, False),
                        (stepw["w_hn"], nx, 1, True)]:
                    mm = nc.tensor.matmul(
                        bank[X][:, half * BST:(half + 1) * BST],
                        w[:], delta_prev[X][:],
                        start=False, stop=stop)
                    if first is None:
                        first = mm
            return first

        for k in range(steps):
            mode = 0 if k == 0 else (1 if k == 1 else 2)

            # Anti-phase ladder: emit [gates_A, sigma_A, gates_B, sigma_B]
            # with gates_B additionally depending on sigma_A (same step) and
            # gates_A on sigma_B (previous step).  Without this the two
            # stream pipelines bunch up in phase and serialize on the shared
            # engines (bus-bunching); the ladder pins a half-step offset.
            ru = [wpool.tile([128, 2 * BST], ew_dt, name=f"ru{X}", bufs=4)
                  for X in range(NST)]
            tt = [wpool.tile([128, BST], ew_dt, name=f"t{X}", bufs=4)
                  for X in range(NST)]
            ts = [wpool.tile([128, BST], ew_dt, name=f"s{X}", bufs=4)
                  for X in range(NST)]
            old_delta = delta_prev
            dj = wpool.tile([128, BSH], mm_dt, name="dj", bufs=4)
            new_delta = [dj[:, 0:BST], dj[:, BST:2 * BST], dj]
            for X in range(NST):
                # ---- full per-stream chain: every engine's FIFO sees
                # [A-op, B-op, A-op, ...] so anti-phased streams never
                # head-of-line block each other. ----
                mm_first = emit_gates(k, mode, X)
                if mm_first is not None:
                    other = stagger.get("s_prev" if X == 0 else "s_cur")
                    if other is not None:
                        add_dep_helper(mm_first.ins, other.ins,
                                       reason="anti-phase ladder")
                nc.scalar.activation(ru[X][:], rz[X][:], Act.Sigmoid)
                nc.vector.tensor_tensor(tt[X][:], nx[X][:, BST:2 * BST],
                                        ru[X][:, 0:BST], Op.mult)
                sop = nc.vector.tensor_tensor(ts[X][:], tt[X][:],
                                              nx[X][:, 0:BST], Op.add)
                stagger["s_cur" if X == 0 else "s_prev"] = sop
                n_ = wpool.tile([128, BST], ew_dt, name=f"n{X}", bufs=4)
                nc.scalar.activation(n_[:], ts[X][:], Act.Tanh)
                dd = wpool.tile([128, BST], ew_dt, name=f"dd{X}", bufs=4)
                nc.vector.tensor_tensor(dd[:], n_[:],
                                        h[:, X * BST:(X + 1) * BST],
                                        Op.subtract)
                nc.vector.tensor_tensor(new_delta[X],
                                        ru[X][:, BST:2 * BST],
                                        dd[:], Op.mult)
                nc.gpsimd.tensor_tensor(h[:, X * BST:(X + 1) * BST],
                                        h[:, X * BST:(X + 1) * BST],
                                        new_delta[X], Op.add)
                nc.tensor.matmul(scratch[:], stepw["w_r"][:],
                                 dummy_src[:], start=True, stop=True)
            delta_prev = new_delta

            # pos matmul + snapshot for the previous step (off-chain tail)
            if old_delta[0] is not None:
                emit_pos(old_delta[2])
                emit_snap(k - 1)
            # PE warm-up fillers: raise TensorE duty so the HAM clock gate
            # holds K=8/8 (2.4 GHz) instead of throttling to 4/8.
            nc.tensor.matmul(scratch[:], stepw["w_r"][:], dummy_src[:],
                             start=True, stop=True)
            nc.tensor.matmul(scratch[:], stepw["w_z"][:], dummy_src[:],
                             start=True, stop=True)

        # trailing pos output for the final step
        emit_pos(delta_prev[2])
        emit_snap(steps - 1)

        # ---- final transpose pass: stage [t, i, b] -> out [b, t, i] ----
        stage_flat = stage[:].rearrange("t p b -> (t p) b")   # [steps*3, BSH]
        TT = 128                                              # t-chunk
        for tc_i in range(steps // TT):
            for c in range(BSH // 128):
                ob = wpool.tile([128, TT * IN], f32, name="ob", bufs=2)
                for r in range(IN):
                    tin = wpool.tile([128, 128], f32, name="tin", bufs=4)
                    nc.sync.dma_start(
                        out=tin[:],
                        in_=stage_flat[tc_i * TT * IN + r * 128:
                                       tc_i * TT * IN + (r + 1) * 128,
                                       c * 128:(c + 1) * 128])
                    tp = ppool.tile([128, 128], f32, name="tp_ps", bufs=2)
                    nc.tensor.transpose(tp[:], tin[:], sb["ident"][:])
                    # source rows r*128..(r+1)*128 are (t*3+i) indices; they
                    # land at the same linear offset in the [b, (t i)] block.
                    nc.vector.tensor_copy(ob[:, r * 128:(r + 1) * 128], tp[:])
                nc.sync.dma_start(
                    out=p_out[c * 128:(c + 1) * 128,
                              tc_i * TT:(tc_i + 1) * TT, :].rearrange(
                        "b t p -> b (t p)"),
                    in_=ob[:])

    nc.finalize()
    return nc


def _get_nc(steps):
    key = (steps, MM_DTYPE, EW_DTYPE)
    if key not in _CACHE:
        _CACHE[key] = _build(steps)
    return _CACHE[key]


def kernel(context, z, steps, W_ih, W_hh, b_ih, b_hh, W_out, b_out):
    from concourse.bass_utils import run_bass_kernel_spmd

    context = np.asarray(context, dtype=np.float32)
    z = np.asarray(z, dtype=np.float32)
    W_ih = np.asarray(W_ih, dtype=np.float32)
    W_hh = np.asarray(W_hh, dtype=np.float32)
    b_ih = np.asarray(b_ih, dtype=np.float32)
    b_hh = np.asarray(b_hh, dtype=np.float32)
    W_out = np.asarray(W_out, dtype=np.float32)
    b_out = np.asarray(b_out, dtype=np.float32)
    steps = int(steps)
    assert context.shape == (B, H) and z.shape == (B, LAT)
    assert steps % TSNAP == 0, steps

    nc = _get_nc(steps)
    in_maps = _host_prep(context, z, W_ih, W_hh, b_ih, b_hh, W_out, b_out)
    res = run_bass_kernel_spmd(nc, in_maps, core_ids=list(range(NCORES)))
    out = np.concatenate([res.results[c]["out"] for c in range(NCORES)], axis=0)
    return out


# revision 58
# speedup vs baseline: 1.0015x; 1.0015x over previous
"""Trainium2 Bass kernel for the autoregressive GRU decoder.

Problem: 512 sequential GRU steps over batch 4096, hidden 128; per step the
3-dim position output feeds back into the input.  Data-parallel over 8 cores
(512 batch rows per core), with the per-core batch split into 2 streams of
256 so the engines can pipeline across the sequential dependency chain.

Algebraic restructuring done on the host (validated vs fp64 golden):
  - pos_k = W_out h_k + b_out feeds the next step's input, so the input
    matmul folds into the hidden one: W_c = W_ih[:, :3] @ W_out + W_hh, with
    all z / bias contributions collapsed into one constant C per gate.
  - The gate pre-activations live persistently in PSUM.  They are
    initialized once with C (via an identity matmul) + W @ h, and every
    subsequent step only accumulates W_c @ delta where
    delta = h_new - h = (u - 1) * (h - n).  No per-step constant re-adds.
  - pos is likewise delta-accumulated, transposed: posT[3, 512] persistent
    in PSUM, += W_out @ delta each step (tiny constant stationary operand),
    snapshotted [3, 512] per step into SBUF and staged to DRAM; a final
    on-chip transpose pass (TensorE) produces the [b, t, i] output layout.

Layout per core: hidden state transposed hT [128(H), 512(B)] in SBUF, fp32.
PSUM banks: rz_A, nx_A, rz_B, nx_B (gate preacts, [r|z] / [xn|hn] halves),
posT, 2 transpose banks.
"""

import os
import numpy as np

B, H, LAT, IN = 4096, 128, 32, 3
NCORES = 8
BSH = B // NCORES          # 512 batch rows per core
NST = 2                    # streams per core
BST = BSH // NST           # 256 batch rows per stream
TSNAP = 16                 # pos snapshots buffered in SBUF between DMAs

# "f32" | "f32r" | "bf16" for the per-step delta matmuls (gates + pos);
# the init matmuls always run exact f32.
MM_DTYPE = os.environ.get("KERNEL_MM_DTYPE", "bf16")
# elementwise working dtype for t/s/n/ru/dd tiles: "f32" | "bf16"
EW_DTYPE = os.environ.get("KERNEL_EW_DTYPE", "bf16")

_CACHE = {}


def _host_prep(context, z, W_ih, W_hh, b_ih, b_hh, W_out, b_out):
    """Fold weights/constants; build per-core input maps."""
    f32 = np.float32
    sl = [slice(0, H), slice(H, 2 * H), slice(2 * H, 3 * H)]  # r, z, n rows

    Wp = (W_ih[:, :IN] @ W_out).astype(f32)           # pos feedback fold
    C0 = (W_ih[:, IN:] @ z.T + b_ih[:, None]).astype(f32)     # [384, B]
    C1 = (C0 + (W_ih[:, :IN] @ b_out)[:, None]).astype(f32)

    Wc_r = (Wp[sl[0]] + W_hh[sl[0]]).astype(f32)
    Wc_z = (Wp[sl[1]] + W_hh[sl[1]]).astype(f32)
    Wc_xn = Wp[sl[2]].astype(f32)
    W_hn = W_hh[sl[2]].astype(f32)

    def asc(a):
        return np.ascontiguousarray(a, dtype=f32)

    weights = {
        "w_r": asc(Wc_r.T), "w_z": asc(-Wc_z.T),
        "w_xn": asc(Wc_xn.T), "w_hn": asc(W_hn.T),
        "w0_r": asc(W_hh[sl[0]].T), "w0_z": asc(-W_hh[sl[1]].T),
        "w_out_t": asc(W_out.T),                       # [128, 3]
        "ident": np.eye(H, dtype=f32),
        "c_pos": asc(np.broadcast_to(b_out[:, None], (IN, BSH))),
    }

    in_maps = []
    for c in range(NCORES):
        bs = slice(c * BSH, (c + 1) * BSH)
        m = dict(weights)
        m["h0t"] = asc(context[bs].T)                  # [H, BSH]
        for X in range(NST):
            cs = slice(c * BSH + X * BST, c * BSH + (X + 1) * BST)
            bhh = b_hh[:, None]
            m[f"c0_rz_{X}"] = asc(np.concatenate(
                [C0[sl[0], cs] + bhh[sl[0]],
                 -(C0[sl[1], cs] + bhh[sl[1]])], axis=1))
            m[f"c1_rz_{X}"] = asc(np.concatenate(
                [C1[sl[0], cs] + bhh[sl[0]],
                 -(C1[sl[1], cs] + bhh[sl[1]])], axis=1))
            hn_const = np.broadcast_to(b_hh[sl[2]][:, None], (H, BST))
            m[f"c0_nx_{X}"] = asc(np.concatenate([C0[sl[2], cs], hn_const], axis=1))
            m[f"c1_nx_{X}"] = asc(np.concatenate([C1[sl[2], cs], hn_const], axis=1))
        in_maps.append(m)
    return in_maps


def _build(steps):
    import concourse.bacc as bacc
    import concourse.mybir as mybir
    from concourse.tile import TileContext

    f32 = mybir.dt.float32
    bf16 = mybir.dt.bfloat16
    Act = mybir.ActivationFunctionType
    Op = mybir.AluOpType

    ew_dt = bf16 if EW_DTYPE == "bf16" else f32
    # dtype for the per-step delta matmul operands (weights + delta).
    mm_dt = {"f32": f32, "f32r": mybir.dt.float32r, "bf16": bf16}[MM_DTYPE]

    nc = bacc.Bacc("TRN2", target_bir_lowering=False, debug=False)

    # ---- DRAM parameters ----
    names_2d = ["h0t"] + [f"c{i}_{g}_{X}" for i in (0, 1)
                          for g in ("rz", "nx") for X in range(NST)]
    params = {}
    for n in names_2d:
        params[n] = nc.declare_dram_parameter(n, [H, BSH], f32, isOutput=False)
    for n in ("w_r", "w_z", "w_xn", "w_hn", "w0_r", "w0_z", "ident"):
        params[n] = nc.declare_dram_parameter(n, [H, H], f32, isOutput=False)
    params["w_out_t"] = nc.declare_dram_parameter("w_out_t", [H, IN], f32,
                                                  isOutput=False)
    params["c_pos"] = nc.declare_dram_parameter("c_pos", [IN, BSH], f32,
                                                isOutput=False)
    p_out = nc.declare_dram_parameter("out", [BSH, steps, IN], f32,
                                      isOutput=True)
    # staging in DRAM for posT snapshots, [t, i, b]
    stage = nc.dram_tensor("pos_stage", [steps, IN, BSH], f32)

    with TileContext(nc) as tc, \
            tc.tile_pool(name="const", bufs=1) as cpool, \
            tc.tile_pool(name="state", bufs=1) as spool, \
            tc.tile_pool(name="work", bufs=3) as wpool, \
            tc.tile_pool(name="stage", bufs=2) as stpool, \
            tc.tile_pool(name="psum", bufs=1, space="PSUM") as ppool:

        sb = {}
        for n, p in params.items():
            t = cpool.tile(list(p.shape), f32, name=f"sb_{n}")
            nc.sync.dma_start(out=t[:], in_=p[:])
            sb[n] = t

        # per-step matmul weights, converted to the mm dtype once
        stepw = {}
        for n in ("w_r", "w_z", "w_xn", "w_hn", "w_out_t"):
            if MM_DTYPE != "f32":
                t = cpool.tile(list(params[n].shape), mm_dt, name=f"cw_{n}")
                nc.vector.tensor_copy(t[:], sb[n][:])
                stepw[n] = t
            else:
                stepw[n] = sb[n]

        h = spool.tile([H, BSH], f32, name="h")
        nc.sync.dma_start(out=h[:], in_=params["h0t"][:])

        rz = [ppool.tile([128, 2 * BST], f32, name=f"ps_rz{X}")
              for X in range(NST)]
        nx = [ppool.tile([128, 2 * BST], f32, name=f"ps_nx{X}")
              for X in range(NST)]
        post = ppool.tile([IN, BSH], f32, name="ps_post")
        # scratch bank + const source for PE warm-up filler matmuls
        scratch = ppool.tile([128, 512], f32, name="ps_scratch")
        dummy_src = cpool.tile([128, 512], mm_dt, name="dummy_src")
        nc.vector.tensor_copy(dummy_src[:], sb["h0t"][:])

        # posT init: b_out + W_out h_0   (exact f32)
        nc.tensor.matmul(post[:], sb["ident"][0:IN, 0:IN], sb["c_pos"][:],
                         start=True, stop=False)
        nc.tensor.matmul(post[:], sb["w_out_t"][:], h[:],
                         start=False, stop=True)

        delta_prev = [None, None]
        stg = [None]
        stagger = {}

        def emit_pos(dja):
            """posT += W_out @ delta_joint (one N=512 matmul; pos is off the
            recurrence critical path so cross-stream coupling is harmless)."""
            nc.tensor.matmul(post[:], stepw["w_out_t"][:], dja[:],
                             start=False, stop=True)

        def emit_snap(k):
            """Snapshot posT (state of step k) into the SBUF staging buffer
            and DMA a TSNAP-step block out when full."""
            kloc = k % TSNAP
            if kloc == 0:
                stg[0] = stpool.tile([IN, TSNAP * BSH], f32, name="stg")
            nc.vector.tensor_copy(stg[0][:, kloc * BSH:(kloc + 1) * BSH],
                                  post[:])
            if kloc == TSNAP - 1:
                t0 = k - kloc
                nc.sync.dma_start(
                    out=stage[t0:t0 + TSNAP, :, :].rearrange(
                        "t p b -> p t b"),
                    in_=stg[0][:].rearrange("p (t b) -> p t b", t=TSNAP))

        from concourse.tile_rust import add_dep_helper

        def emit_gates(k, mode, X):
            """Gate matmuls + pos for one stream; returns the first matmul
            (for the anti-phase dependency ladder)."""
            first = None
            if mode < 2:
                ci = f"c{mode}"
                first = nc.tensor.matmul(rz[X][:], sb["ident"][:],
                                         sb[f"{ci}_rz_{X}"][:],
                                         start=True, stop=False)
                nc.tensor.matmul(nx[X][:], sb["ident"][:],
                                 sb[f"{ci}_nx_{X}"][:],
                                 start=True, stop=(mode == 0))
                wr = sb["w0_r"] if mode == 0 else sb["w_r"]
                wz = sb["w0_z"] if mode == 0 else sb["w_z"]
                gate_mms = [(wr, rz, 0, False), (wz, rz, 1, True),
                            (sb["w_hn"], nx, 1, True)]
                if mode == 1:
                    gate_mms.insert(2, (sb["w_xn"], nx, 0, False))
                for w, bank, half, stop in gate_mms:
                    nc.tensor.matmul(
                        bank[X][:, half * BST:(half + 1) * BST],
                        w[:], h[:, X * BST:(X + 1) * BST],
                        start=False, stop=stop)
            else:
                for w, bank, half, stop in [
                        (stepw["w_r"], rz, 0, False),
                        (stepw["w_z"], rz, 1, True),
                        (stepw["w_xn"], nx, 0, False),
                        (stepw["w_hn"], nx, 1, True)]:
                    mm = nc.tensor.matmul(
                        bank[X][:, half * BST:(half + 1) * BST],
                        w[:], delta_prev[X][:],
                        start=False, stop=stop)
                    if first is None:
                        first = mm
            return first

        for k in range(steps):
            mode = 0 if k == 0 else (1 if k == 1 else 2)

            # Anti-phase ladder: emit [gates_A, sigma_A, gates_B, sigma_B]
            # with gates_B additionally depending on sigma_A (same step) and
            # gates_A on sigma_B (previous step).  Without this the two
            # stream pipelines bunch up in phase and serialize on the shared
            # engines (bus-bunching); the ladder pins a half-step offset.
            ru = [wpool.tile([128, 2 * BST], ew_dt, name=f"ru{X}", bufs=4)
                  for X in range(NST)]
            tt = [wpool.tile([128, BST], ew_dt, name=f"t{X}", bufs=4)
                  for X in range(NST)]
            ts = [wpool.tile([128, BST], ew_dt, name=f"s{X}", bufs=4)
                  for X in range(NST)]
            old_delta = delta_prev
            dj = wpool.tile([128, BSH], mm_dt, name="dj", bufs=4)
            new_delta = [dj[:, 0:BST], dj[:, BST:2 * BST], dj]
            for X in range(NST):
                # ---- full per-stream chain: every engine's FIFO sees
                # [A-op, B-op, A-op, ...] so anti-phased streams never
                # head-of-line block each other. ----
                mm_first = emit_gates(k, mode, X)
                if mm_first is not None:
                    other = stagger.get("s_prev" if X == 0 else "s_cur")
                    if other is not None:
                        add_dep_helper(mm_first.ins, other.ins,
                                       reason="anti-phase ladder")
                nc.scalar.activation(ru[X][:], rz[X][:], Act.Sigmoid)
                nc.vector.tensor_tensor(tt[X][:], nx[X][:, BST:2 * BST],
                                        ru[X][:, 0:BST], Op.mult)
                sop = nc.vector.tensor_tensor(ts[X][:], tt[X][:],
                                              nx[X][:, 0:BST], Op.add)
                stagger["s_cur" if X == 0 else "s_prev"] = sop
                n_ = wpool.tile([128, BST], ew_dt, name=f"n{X}", bufs=4)
                nc.scalar.activation(n_[:], ts[X][:], Act.Tanh)
                dd = wpool.tile([128, BST], ew_dt, name=f"dd{X}", bufs=4)
                nc.vector.tensor_tensor(dd[:], n_[:],
                                        h[:, X * BST:(X + 1) * BST],
                                        Op.subtract)
                nc.vector.tensor_tensor(new_delta[X],
                                        ru[X][:, BST:2 * BST],
                                        dd[:], Op.mult)
                nc.gpsimd.tensor_tensor(h[:, X * BST:(X + 1) * BST],
                                        h[:, X * BST:(X + 1) * BST],
                                        new_delta[X], Op.add)
                nc.tensor.matmul(scratch[:], stepw["w_r"][:],
                                 dummy_src[:], start=True, stop=True)
            delta_prev = new_delta

            # pos matmul + snapshot for the previous step (off-chain tail)
            if old_delta[0] is not None:
                emit_pos(old_delta[2])
                emit_snap(k - 1)
            # PE warm-up filler: raise TensorE duty so the HAM clock gate
            # holds K=8/8 (2.4 GHz) instead of throttling to 4/8.
            nc.tensor.matmul(scratch[:], stepw["w_r"][:], dummy_src[:],
                             start=True, stop=True)

        # trailing pos output for the final step
        emit_pos(delta_prev[2])
        emit_snap(steps - 1)

        # ---- final transpose pass: stage [t, i, b] -> out [b, t, i] ----
        stage_flat = stage[:].rearrange("t p b -> (t p) b")   # [steps*3, BSH]
        TT = 128                                              # t-chunk
        for tc_i in range(steps // TT):
            for c in range(BSH // 128):
                ob = wpool.tile([128, TT * IN], f32, name="ob", bufs=2)
                for r in range(IN):
                    tin = wpool.tile([128, 128], f32, name="tin", bufs=4)
                    nc.sync.dma_start(
                        out=tin[:],
                        in_=stage_flat[tc_i * TT * IN + r * 128:
                                       tc_i * TT * IN + (r + 1) * 128,
                                       c * 128:(c + 1) * 128])
                    tp = ppool.tile([128, 128], f32, name="tp_ps", bufs=2)
                    nc.tensor.transpose(tp[:], tin[:], sb["ident"][:])
                    # source rows r*128..(r+1)*128 are (t*3+i) indices; they
                    # land at the same linear offset in the [b, (t i)] block.
                    nc.vector.tensor_copy(ob[:, r * 128:(r + 1) * 128], tp[:])
                nc.sync.dma_start(
                    out=p_out[c * 128:(c + 1) * 128,
                              tc_i * TT:(tc_i + 1) * TT, :].rearrange(
                        "b t p -> b (t p)"),
                    in_=ob[:])

    nc.finalize()
    return nc


def _get_nc(steps):
    key = (steps, MM_DTYPE, EW_DTYPE)
    if key not in _CACHE:
        _CACHE[key] = _build(steps)
    return _CACHE[key]


def kernel(context, z, steps, W_ih, W_hh, b_ih, b_hh, W_out, b_out):
    from concourse.bass_utils import run_bass_kernel_spmd

    context = np.asarray(context, dtype=np.float32)
    z = np.asarray(z, dtype=np.float32)
    W_ih = np.asarray(W_ih, dtype=np.float32)
    W_hh = np.asarray(W_hh, dtype=np.float32)
    b_ih = np.asarray(b_ih, dtype=np.float32)
    b_hh = np.asarray(b_hh, dtype=np.float32)
    W_out = np.asarray(W_out, dtype=np.float32)
    b_out = np.asarray(b_out, dtype=np.float32)
    steps = int(steps)
    assert context.shape == (B, H) and z.shape == (B, LAT)
    assert steps % TSNAP == 0, steps

    nc = _get_nc(steps)
    in_maps = _host_prep(context, z, W_ih, W_hh, b_ih, b_hh, W_out, b_out)
    res = run_bass_kernel_spmd(nc, in_maps, core_ids=list(range(NCORES)))
    out = np.concatenate([res.results[c]["out"] for c in range(NCORES)], axis=0)
    return out
